# revision 24
# baseline (speedup 1.0000x reference)
"""AudioFinder Trainium2 kernel.

Data parallel over batch: 16 samples -> 8 cores x 2 samples.

Per-core pipeline (bf16 matmuls / f32 psum, both samples interleaved
layer-by-layer so one sample's matmuls fill the other's pipeline-latency
bubbles on the in-order engine queues):
  1. Both query encoders (T=2048 -> 504), layers interleaved; v =
     w_lin @ enc_q; the tiled-x4 + pad row [1,2048] is bounced through
     DRAM into [128,16] (t = p + 128*b) while the search encoders run.
  2. Both search encoders (T=8192 -> 2040), layers interleaved.  The
     f0/f1 head + VQ are fused into layer 3's per-chunk pipeline so the
     VQ DVE reductions spread across the conv matmul span.
  3. VQ per 128-t block: three bf16 matmuls (enc block stationary)
     against epk3's three NK-column blocks, DVE max-reduce each:
     scores s[t,k] = enc_s[t]@emb[k] - |emb[k]|^2/2 (enc rows 80/81
     const 1.0; epk3 row 80 carries -|e|^2/2 shared, row 81 carries
     0 / ew0/BIG / ew1/BIG with ew = emb @ w_lin.T):
       u_j[t] = max_k (s[t,k] + ew[k,j]/BIG),  m[t] = max_k s[t,k]
       => (u_j - m)*BIG = ew[argmax_k s, j]   (fp32-psum exact)
  4. z = (u-m)*BIG + vt in [128,16]; max over free dim on DVE, across
     partitions on Pool; out = tanh(max z + b_lin).

Conv layers: 4 taps as PSUM-accumulated matmuls over Cin=80, gated
tanh*sigmoid on ACT engine, gate product on Pool, 1x1 conv + residual
writes on DVE.  wpk is packed layer-major and DMA'd in two pieces so
the first matmul only waits for layer 0's weights.
"""

import numpy as np
import ml_dtypes

import concourse.bacc as bacc
import concourse.mybir as mybir
import concourse.tile as tile
from concourse.bass_utils import run_bass_kernel_spmd

F32 = mybir.dt.float32
BF16 = mybir.dt.bfloat16
AF = mybir.ActivationFunctionType
OP = mybir.AluOpType
AX = mybir.AxisListType

NCORES = 8
SPC = 2          # samples per core
C = 80
NK = 512         # codebook size
BIG = 1024.0
NEG = -1e30
CH = 512         # chunk (free-dim) size

# layer geometry
GEO_SEARCH = dict(T0h=4096, T1=4095, E1=2048, O1=2047, T2=2046, T3=2043, T4=2040)
GEO_QUERY = dict(T0h=1024, T1=1023, E1=512, O1=511, T2=510, T3=507, T4=504)

# wpack layout: layer-major [a_i(4 taps), g_i(4 taps), w1x1_i] x 4, f0, f1
LBLK = 720  # 4*80 + 4*80 + 80

def _w_off(kind, i, j=0):
    if kind == "a":
        return LBLK * i + C * j
    if kind == "g":
        return LBLK * i + 320 + C * j
    if kind == "1":
        return LBLK * i + 640
    if kind == "f0":
        return 2880
    if kind == "f1":
        return 2960
    raise KeyError(kind)


M_F1 = 82  # f1 conv emits 80 real channels + two const-1 channels


WPACK_COLS = 3042
# bias pack columns: ba0..3, bg0..3, b10..3, bf0, bf1
def _b_off(kind, i=0):
    return {"a": i, "g": 4 + i, "1": 8 + i, "f0": 12, "f1": 13}[kind]


def _build():
    nc = bacc.Bacc("TRN2", target_bir_lowering=False, debug=False,
                   num_devices=NCORES)
    d_se = nc.dram_tensor("se", [SPC, 128, 4096], BF16, kind="ExternalInput")
    d_so = nc.dram_tensor("so", [SPC, 128, 4096], BF16, kind="ExternalInput")
    d_qe = nc.dram_tensor("qe", [SPC, 128, 1024], BF16, kind="ExternalInput")
    d_qo = nc.dram_tensor("qo", [SPC, 128, 1024], BF16, kind="ExternalInput")
    d_wpk = nc.dram_tensor("wpk", [128, WPACK_COLS], BF16, kind="ExternalInput")
    d_bpk = nc.dram_tensor("bpk", [M_F1, 14], F32, kind="ExternalInput")
    d_epk3 = nc.dram_tensor("epk3", [128, 3 * NK], BF16, kind="ExternalInput")
    d_z = nc.dram_tensor("z48", [48, 4104], BF16, kind="ExternalInput")
    d_wlt = nc.dram_tensor("wlt", [C, 2], BF16, kind="ExternalInput")
    d_blt = nc.dram_tensor("blt", [1, 4], F32, kind="ExternalInput")
    d_out = nc.dram_tensor("out", [1, 4], F32, kind="ExternalOutput")
    d_zbuf = nc.dram_tensor("zbuf", [2 * SPC, 2048], F32)

    with tile.TileContext(nc) as tc:
        with (
            tc.tile_pool(name="sb", bufs=1) as sb,
            tc.tile_pool(name="ps", bufs=2, space="PSUM") as ps,
        ):
            # ---- startup DMAs, ordered so layer-0 work can start early ----
            wpk = sb.tile([128, WPACK_COLS], BF16, tag="wpk")
            nc.sync.dma_start(wpk[:, :LBLK], d_wpk[:, :LBLK])
            bpk = sb.tile([M_F1, 14], F32, tag="bpk")
            nc.sync.dma_start(bpk[:], d_bpk[:])

            def wsl(kind, i, j=0, rows=128):
                off = _w_off(kind, i, j)
                return wpk[:rows, off:off + C]

            def bap(kind, i=0):
                o = _b_off(kind, i)
                n = M_F1 if kind == "f1" else C
                return bpk[:n, o:o + 1]

            def wide_chunk(i, taps, T_out, write_out, c0, mul_pool=False):
                """One CH-column chunk of wide conv layer i."""
                N = min(CH, T_out - c0)
                Nmm = N + (N & 1)
                aps = ps.tile([C, Nmm], F32, tag="aps")
                gps = ps.tile([C, Nmm], F32, tag="gps")
                for half, pt in (("a", aps), ("g", gps)):
                    for j, (src, off) in enumerate(taps):
                        nc.tensor.matmul(
                            pt[:], wsl(half, i, j),
                            src[:, off + c0: off + c0 + Nmm],
                            start=(j == 0), stop=(j == 3))
                ta = sb.tile([C, Nmm], BF16, tag="ta", bufs=2)
                sg = sb.tile([C, Nmm], BF16, tag="sg", bufs=2)
                nc.scalar.activation(ta[:], aps[:], AF.Tanh, bias=bap("a", i))
                nc.scalar.activation(sg[:], gps[:], AF.Sigmoid, bias=bap("g", i))
                x2 = sb.tile([C, Nmm], BF16, tag="x2", bufs=2)
                nc.gpsimd.tensor_mul(x2[:], ta[:], sg[:])
                xps = ps.tile([C, Nmm], F32, tag="xps")
                nc.tensor.matmul(xps[:], wsl("1", i, rows=C), x2[:],
                                 start=True, stop=True)
                write_out(c0, N, xps[:, :N])

            def encoder_units(s, g, is_query, vq_cb=None):
                """Chunk-level emitters for the scheduler.

                Layer 3 fuses the f0/f1 head (+VQ for search)."""
                T0h, T1 = g["T0h"], g["T1"]
                E1, O1 = g["E1"], g["O1"]
                T2, T3, T4 = g["T2"], g["T3"], g["T4"]
                d_e, d_o = (d_qe, d_qo) if is_query else (d_se, d_so)
                sfx = f"{'q' if is_query else 's'}{s}"
                st = {}

                def load():
                    x0e = sb.tile([128, T0h + 8], BF16, tag=f"x0e{sfx}")
                    x0o = sb.tile([128, T0h + 8], BF16, tag=f"x0o{sfx}")
                    nc.vector.memset(x0e[:, T0h:], 0.0)
                    nc.vector.memset(x0o[:, T0h:], 0.0)
                    for eng, dst, dsrc in ((nc.sync, x0e, d_e),
                                           (nc.scalar, x0o, d_o)):
                        for c0 in range(0, T0h, 2048):
                            n = min(2048, T0h - c0)
                            eng.dma_start(dst[:, c0:c0 + n],
                                          dsrc[s, :, c0:c0 + n])
                    st.update(x0e=x0e, x0o=x0o)

                def alloc():
                    x1e = sb.tile([128, E1 + 8], BF16, tag=f"x1e{sfx}")
                    x1o = sb.tile([128, O1 + 8], BF16, tag=f"x1o{sfx}")
                    nc.vector.memset(x1e[:, E1:], 0.0)
                    nc.vector.memset(x1o[:, O1:], 0.0)
                    nc.scalar.dma_start(x1e[C:, :E1], d_z[:, :E1])
                    nc.scalar.dma_start(x1o[C:, :O1], d_z[:, :O1])
                    x2f = sb.tile([128, T2 + 8], BF16, tag=f"x2f{sfx}")
                    nc.vector.memset(x2f[:, T2:], 0.0)
                    nc.scalar.dma_start(x2f[C:, :T2], d_z[:, :T2])
                    x3f = sb.tile([128, T3 + 8], BF16, tag=f"x3f{sfx}")
                    nc.vector.memset(x3f[:, T3:], 0.0)
                    nc.scalar.dma_start(x3f[C:, :T3], d_z[:, :T3])
                    x4f = sb.tile([128, T4 + 8], BF16, tag=f"x4f{sfx}")
                    nc.vector.memset(x4f[:, T4:], 0.0)
                    nc.scalar.dma_start(x4f[C:, :T4], d_z[:, :T4])
                    # f1(relu(f0(x))); rows 80/81 of enc are const 1.0
                    # (zero weights, bias 1) for the VQ score offsets.
                    enc = sb.tile([128, T4], BF16, tag=f"enc{sfx}")
                    nc.scalar.dma_start(enc[M_F1:, :], d_z[:128 - M_F1, :T4])
                    st.update(x1e=x1e, x1o=x1o, x2f=x2f,
                              x3f=x3f, x4f=x4f, enc=enc)

                def w0(c0, N, xps):
                    ne, no = (N + 1) // 2, N // 2
                    h = c0 // 2
                    nc.scalar.activation(
                        st["x1e"][:C, h:h + ne], xps[:, 0:N:2], AF.Identity,
                        bias=bap("1", 0))
                    nc.vector.tensor_scalar(
                        st["x1o"][:C, h:h + no], xps[:, 1:N:2], bap("1", 0),
                        None, op0=OP.add)

                def w1(c0, N, xps):
                    nc.vector.scalar_tensor_tensor(
                        out=st["x2f"][:C, c0:c0 + N], in0=xps,
                        scalar=bap("1", 1),
                        in1=st["x1o"][:C, c0 + 1:c0 + 1 + N],
                        op0=OP.add, op1=OP.add)

                def w2(c0, N, xps):
                    nc.vector.scalar_tensor_tensor(
                        out=st["x3f"][:C, c0:c0 + N], in0=xps,
                        scalar=bap("1", 2),
                        in1=st["x2f"][:C, c0 + 3:c0 + 3 + N],
                        op0=OP.add, op1=OP.add)

                def w3(c0, N, xps):
                    # keep the x4f write off DVE (VQ reduces queue there):
                    # ACT applies the 1x1 bias, Pool adds the residual.
                    x4f, x3f, enc = st["x4f"], st["x3f"], st["enc"]
                    x4t = sb.tile([C, N], BF16, tag="x4t", bufs=2)
                    nc.scalar.activation(x4t[:], xps, AF.Identity,
                                         bias=bap("1", 3))
                    nc.gpsimd.tensor_add(x4f[:C, c0:c0 + N], x4t[:],
                                         x3f[:C, c0 + 3:c0 + 3 + N])
                    p0 = ps.tile([C, N], F32, tag="xps")
                    nc.tensor.matmul(p0[:], wsl("f0", 0), x4f[:, c0:c0 + N],
                                     start=True, stop=True)
                    xf = sb.tile([C, N], BF16, tag="xf", bufs=2)
                    nc.scalar.activation(xf[:], p0[:], AF.Relu, bias=bap("f0"))
                    p1 = ps.tile([M_F1, N], F32, tag="xps")
                    nc.tensor.matmul(p1[:], wpk[:C, 2960:2960 + M_F1], xf[:],
                                     start=True, stop=True)
                    nc.scalar.activation(enc[:M_F1, c0:c0 + N], p1[:],
                                         AF.Identity, bias=bap("f1"))
                    if vq_cb is not None:
                        vq_cb(c0, N, enc)

                Ts = [T1, T2, T3, T4]
                writers = [w0, w1, w2, w3]

                def taps(i):
                    if i == 0:
                        return [(st["x0e"], 0), (st["x0o"], 0),
                                (st["x0e"], 1), (st["x0o"], 1)]
                    if i == 1:
                        return [(st["x1e"], 0), (st["x1o"], 0),
                                (st["x1e"], 1), (st["x1o"], 1)]
                    x = st["x2f"] if i == 2 else st["x3f"]
                    return [(x, 0), (x, 1), (x, 2), (x, 3)]

                def emit(i, c):
                    wide_chunk(i, taps(i), Ts[i], writers[i], c * CH,
                               mul_pool=(i == 3))

                nch = [-(-t // CH) for t in Ts]
                ins = [E1, T2, T3]

                def req(i, c):
                    """Chunks of layer i-1 needed before chunk c of layer i."""
                    per = CH // 2 if i == 1 else CH
                    need = min(CH * c + 515, ins[i - 1])
                    return min(nch[i - 1], -(-need // per))

                return dict(load=load, alloc=alloc, emit=emit, nch=nch,
                            req=req, st=st)

            def run_sched(encs):
                """Greedy deepest-ready-first, round-robin over samples."""
                prog = [[0] * 4 for _ in encs]
                remaining = sum(sum(e["nch"]) for e in encs)
                turn = 0
                while remaining:
                    emitted = False
                    for k in range(len(encs)):
                        sidx = (turn + k) % len(encs)
                        e, p = encs[sidx], prog[sidx]
                        for i in (3, 2, 1, 0):
                            if p[i] >= e["nch"][i]:
                                continue
                            if i > 0 and p[i - 1] < e["req"](i, p[i]):
                                continue
                            e["emit"](i, p[i])
                            p[i] += 1
                            remaining -= 1
                            emitted = True
                            break
                        if emitted:
                            break
                    turn += 1
                    assert emitted, "scheduler deadlock"

            # ---- query encoders ----
            qencs = [encoder_units(s, GEO_QUERY, True) for s in range(SPC)]
            for e in qencs:
                e["load"]()        # input DMAs queue first
            for e in qencs:
                e["alloc"]()
            # remaining static tables arrive behind the query inputs
            nc.sync.dma_start(wpk[:, LBLK:], d_wpk[:, LBLK:])
            epk3 = sb.tile([128, 3 * NK], BF16, tag="epk3")
            nc.sync.dma_start(epk3[:], d_epk3[:])
            wlt = sb.tile([C, 2], BF16, tag="wlt")
            nc.sync.dma_start(wlt[:], d_wlt[:])
            brow = sb.tile([1, 4], F32, tag="brow")
            nc.sync.dma_start(brow[:], d_blt[:])
            run_sched(qencs)

            zred = sb.tile([128, 4], F32, tag="zred")
            vts = {}
            for s in range(SPC):
                enc_q = qencs[s]["st"]["enc"]
                vps = ps.tile([2, 504], F32, tag="xps")
                nc.tensor.matmul(vps[:], wlt[:], enc_q[:C, :504],
                                 start=True, stop=True)  # 80-row: tiny
                vrow = sb.tile([2, 2048], F32, tag="vrow", bufs=2)
                for k in range(4):
                    nc.scalar.activation(vrow[:, 504 * k:504 * (k + 1)],
                                         vps[:], AF.Copy)
                nc.vector.memset(vrow[:, 2016:2040], 0.0)
                nc.vector.memset(vrow[:, 2040:2048], NEG)
                nc.sync.dma_start(d_zbuf[2 * s:2 * s + 2, :], vrow[:])
                vt0 = sb.tile([128, 16], F32, tag=f"vt{2 * s}")
                vt1 = sb.tile([128, 16], F32, tag=f"vt{2 * s + 1}")
                nc.sync.dma_start(
                    vt0[:], d_zbuf[2 * s].rearrange("(b p) -> p b", p=128))
                nc.sync.dma_start(
                    vt1[:], d_zbuf[2 * s + 1].rearrange("(b p) -> p b", p=128))
                vts[s] = (vt0, vt1)

            # ---- search encoders with fused VQ, interleaved ----
            T4s = GEO_SEARCH["T4"]
            accs = {}
            for s in range(SPC):
                mt = sb.tile([128, 16], F32, tag=f"mt{s}")
                u0t = sb.tile([128, 16], F32, tag=f"u0t{s}")
                u1t = sb.tile([128, 16], F32, tag=f"u1t{s}")
                nc.vector.memset(mt[:], NEG)
                nc.vector.memset(u0t[:], NEG)
                nc.vector.memset(u1t[:], NEG)
                accs[s] = (mt, u0t, u1t)

            def make_vq_cb(s):
                mt, u0t, u1t = accs[s]

                def vq_blocks(c0, N, enc):
                    b0 = (c0 + 127) // 128
                    b1 = (c0 + N) // 128 if c0 + N < T4s else 16
                    for b in range(b0, b1):
                        t0 = 128 * b
                        P = min(128, T4s - t0)
                        for ti, tgt in ((0, mt), (1, u0t), (2, u1t)):
                            sps = ps.tile([P, NK], F32, tag="vq")
                            nc.tensor.matmul(
                                sps[:], enc[:, t0:t0 + P],
                                epk3[:, NK * ti:NK * (ti + 1)],
                                start=True, stop=True)
                            nc.vector.tensor_reduce(
                                tgt[:P, b:b + 1], sps[:], axis=AX.X,
                                op=OP.max)
                return vq_blocks

            sencs = [encoder_units(s, GEO_SEARCH, False,
                                   vq_cb=make_vq_cb(s))
                     for s in range(SPC)]
            for e in sencs:
                e["load"]()
            for e in sencs:
                e["alloc"]()
            run_sched(sencs)

            # ---- z = (u - m)*BIG + vt, reduce ----
            for s in range(SPC):
                mt, u0t, u1t = accs[s]
                for j, ut in ((0, u0t), (1, u1t)):
                    zt = sb.tile([128, 16], F32, tag="zt", bufs=2)
                    nc.vector.tensor_sub(zt[:], ut[:], mt[:])
                    nc.vector.scalar_tensor_tensor(
                        out=zt[:], in0=zt[:], scalar=BIG, in1=vts[s][j][:],
                        op0=OP.mult, op1=OP.add)
                    nc.vector.tensor_reduce(
                        zred[:, 2 * s + j:2 * s + j + 1], zt[:], axis=AX.X,
                        op=OP.max)

            import concourse.bass_isa as bass_isa
            zar = sb.tile([128, 4], F32, tag="zar")
            nc.gpsimd.partition_all_reduce(zar[:], zred[:], channels=128,
                                           reduce_op=bass_isa.ReduceOp.max)
            zrow = sb.tile([1, 4], F32, tag="zrow")
            nc.vector.tensor_add(zrow[:], zar[0:1, :], brow[:])
            outv = sb.tile([1, 4], F32, tag="outv")
            nc.scalar.activation(outv[:], zrow[:], AF.Tanh)
            nc.sync.dma_start(d_out[:], outv[:])

    nc.finalize()
    return nc


_NC_CACHE = None


def _get_nc():
    global _NC_CACHE
    if _NC_CACHE is None:
        _NC_CACHE = _build()
    return _NC_CACHE


def prep_inputs(search, query, w_wide, b_wide, w_1x1, b_1x1, w_f0, b_f0,
                w_f1, b_f1, embedding, w_lin, b_lin):
    """Host-side packing -> list of per-core input maps (bf16 operands)."""
    f = np.float32
    bf = ml_dtypes.bfloat16
    search = np.asarray(search, f)
    query = np.asarray(query, f)

    def pad128(a):
        # (N, T, C) -> (N, 128, T) channel-major, rows C..127 zero
        n, t, _ = a.shape
        out = np.zeros((n, 128, t), dtype=bf)
        out[:, :C, :] = a.transpose(0, 2, 1).astype(bf)
        return out

    se = pad128(search[:, 0::2, :])
    so = pad128(search[:, 1::2, :])
    qe = pad128(query[:, 0::2, :])
    qo = pad128(query[:, 1::2, :])

    w_wide = np.asarray(w_wide, f)
    cols = []
    for i in range(4):
        for j in range(4):
            cols.append(w_wide[i, :C, :, j].T)     # a taps
        for j in range(4):
            cols.append(w_wide[i, C:, :, j].T)     # g taps
        cols.append(np.asarray(w_1x1, f)[i, :, :, 0].T)
    cols.append(np.asarray(w_f0, f)[:, :, 0].T)
    wf1 = np.zeros((C, M_F1), f)
    wf1[:, :C] = np.asarray(w_f1, f)[:, :, 0].T   # cols 80/81 stay zero
    cols.append(wf1)
    wpk80 = np.ascontiguousarray(np.concatenate(cols, axis=1))
    assert wpk80.shape == (C, WPACK_COLS)
    wpk = np.zeros((128, WPACK_COLS), dtype=bf)
    wpk[:C] = wpk80.astype(bf)

    b_wide = np.asarray(b_wide, f)
    bcols = [b_wide[i, :C] for i in range(4)]
    bcols += [b_wide[i, C:] for i in range(4)]
    bcols += [np.asarray(b_1x1, f)[i] for i in range(4)]
    bcols += [np.asarray(b_f0, f), np.asarray(b_f1, f)]
    bpk = np.zeros((M_F1, 14), f)
    bpk[:C] = np.stack(bcols, axis=1)
    bpk[C, _b_off("f1")] = 1.0     # f1 rows 80/81 = 0*x + 1.0 -> const-1
    bpk[C + 1, _b_off("f1")] = 1.0

    emb = np.asarray(embedding, f)[0]            # (512, 80)
    e2 = (emb.astype(np.float64) ** 2).sum(1)
    ew = (emb.astype(np.float64) @ np.asarray(w_lin, f).T.astype(np.float64))
    epk3 = np.zeros((128, 3 * NK), f)
    for ti in range(3):
        epk3[:C, NK * ti:NK * (ti + 1)] = emb.T
        epk3[C, NK * ti:NK * (ti + 1)] = -0.5 * e2
    epk3[C + 1, NK:2 * NK] = ew[:, 0] / BIG
    epk3[C + 1, 2 * NK:3 * NK] = ew[:, 1] / BIG
    epk3 = epk3.astype(bf)
    z48 = np.zeros((48, 4104), dtype=bf)
    wlt = np.ascontiguousarray(np.asarray(w_lin, f).T).astype(bf)
    b_lin = np.asarray(b_lin, f)
    blt = np.array([[b_lin[0], b_lin[1], b_lin[0], b_lin[1]]], f)

    maps = []
    for c in range(NCORES):
        sl = slice(SPC * c, SPC * (c + 1))
        maps.append({
            "se": se[sl], "so": so[sl], "qe": qe[sl], "qo": qo[sl],
            "wpk": wpk, "bpk": bpk, "epk3": epk3, "wlt": wlt, "blt": blt,
            "z48": z48,
        })
    return maps


def kernel(**inputs):
    nc = _get_nc()
    maps = prep_inputs(**inputs)
    res = run_bass_kernel_spmd(nc, maps, core_ids=list(range(NCORES)))
    out = np.concatenate([r["out"].reshape(SPC, 2) for r in res.results],
                         axis=0)
    return out.astype(np.float32)


if __name__ == "__main__":
    import reference
    inputs = {k: np.asarray(v) for k, v in reference.setup_inputs().items()}
    got = kernel(**inputs)
    print(got)


# revision 25
# speedup vs baseline: 1.1333x; 1.1333x over previous
"""AudioFinder Trainium2 kernel.

Data parallel over batch: 16 samples -> 8 cores x 2 samples.

Per-core pipeline (bf16 matmuls / f32 psum, both samples interleaved
layer-by-layer so one sample's matmuls fill the other's pipeline-latency
bubbles on the in-order engine queues):
  1. Both query encoders (T=2048 -> 504), layers interleaved; v =
     w_lin @ enc_q; the tiled-x4 + pad row [1,2048] is bounced through
     DRAM into [128,16] (t = p + 128*b) while the search encoders run.
  2. Both search encoders (T=8192 -> 2040), layers interleaved.  The
     f0/f1 head + VQ are fused into layer 3's per-chunk pipeline so the
     VQ DVE reductions spread across the conv matmul span.
  3. VQ per 128-t block: three bf16 matmuls (enc block stationary)
     against epk3's three NK-column blocks, DVE max-reduce each:
     scores s[t,k] = enc_s[t]@emb[k] - |emb[k]|^2/2 (enc rows 80/81
     const 1.0; epk3 row 80 carries -|e|^2/2 shared, row 81 carries
     0 / ew0/BIG / ew1/BIG with ew = emb @ w_lin.T):
       u_j[t] = max_k (s[t,k] + ew[k,j]/BIG),  m[t] = max_k s[t,k]
       => (u_j - m)*BIG = ew[argmax_k s, j]   (fp32-psum exact)
  4. z = (u-m)*BIG + vt in [128,16]; max over free dim on DVE, across
     partitions on Pool; out = tanh(max z + b_lin).

Conv layers: 4 taps as PSUM-accumulated matmuls over Cin=80, gated
tanh*sigmoid on ACT engine, gate product on Pool, 1x1 conv + residual
writes on DVE.  wpk is packed layer-major and DMA'd in two pieces so
the first matmul only waits for layer 0's weights.
"""

import numpy as np
import ml_dtypes

import concourse.bacc as bacc
import concourse.mybir as mybir
import concourse.tile as tile
from concourse.bass_utils import run_bass_kernel_spmd

F32 = mybir.dt.float32
BF16 = mybir.dt.bfloat16
AF = mybir.ActivationFunctionType
OP = mybir.AluOpType
AX = mybir.AxisListType

NCORES = 8
SPC = 2          # samples per core
C = 80
NK = 512         # codebook size
BIG = 1024.0
NEG = -1e30
CH = 512         # chunk (free-dim) size

# layer geometry
GEO_SEARCH = dict(T0h=4096, T1=4095, E1=2048, O1=2047, T2=2046, T3=2043, T4=2040)
GEO_QUERY = dict(T0h=1024, T1=1023, E1=512, O1=511, T2=510, T3=507, T4=504)

# wpack layout: layer-major [a_i(4 taps), g_i(4 taps), w1x1_i] x 4, f0, f1
LBLK = 720  # 4*80 + 4*80 + 80

def _w_off(kind, i, j=0):
    if kind == "a":
        return LBLK * i + C * j
    if kind == "g":
        return LBLK * i + 320 + C * j
    if kind == "1":
        return LBLK * i + 640
    if kind == "f0":
        return 2880
    if kind == "f1":
        return 2960
    raise KeyError(kind)


M_F1 = 82  # f1 conv emits 80 real channels + two const-1 channels


WPACK_COLS = 3042
# bias pack columns: ba0..3, bg0..3, b10..3, bf0, bf1
def _b_off(kind, i=0):
    return {"a": i, "g": 4 + i, "1": 8 + i, "f0": 12, "f1": 13}[kind]


def _build():
    nc = bacc.Bacc("TRN2", target_bir_lowering=False, debug=False,
                   num_devices=NCORES)
    d_se = nc.dram_tensor("se", [SPC, 128, 4096], BF16, kind="ExternalInput")
    d_so = nc.dram_tensor("so", [SPC, 128, 4096], BF16, kind="ExternalInput")
    d_qe = nc.dram_tensor("qe", [SPC, 128, 1024], BF16, kind="ExternalInput")
    d_qo = nc.dram_tensor("qo", [SPC, 128, 1024], BF16, kind="ExternalInput")
    d_wpk = nc.dram_tensor("wpk", [128, WPACK_COLS], BF16, kind="ExternalInput")
    d_bpk = nc.dram_tensor("bpk", [M_F1, 14], F32, kind="ExternalInput")
    d_epk3 = nc.dram_tensor("epk3", [128, 3 * NK], BF16, kind="ExternalInput")
    d_z = nc.dram_tensor("z48", [48, 4104], BF16, kind="ExternalInput")
    d_wlt = nc.dram_tensor("wlt", [C, 2], BF16, kind="ExternalInput")
    d_blt = nc.dram_tensor("blt", [1, 4], F32, kind="ExternalInput")
    d_out = nc.dram_tensor("out", [1, 4], F32, kind="ExternalOutput")
    d_zbuf = nc.dram_tensor("zbuf", [2 * SPC, 2048], F32)

    with tile.TileContext(nc) as tc:
        with (
            tc.tile_pool(name="sb", bufs=1) as sb,
            tc.tile_pool(name="ps", bufs=2, space="PSUM") as ps,
        ):
            # ---- startup DMAs, ordered so layer-0 work can start early ----
            wpk = sb.tile([128, WPACK_COLS], BF16, tag="wpk")
            nc.sync.dma_start(wpk[:, :LBLK], d_wpk[:, :LBLK])
            bpk = sb.tile([M_F1, 14], F32, tag="bpk")
            nc.sync.dma_start(bpk[:], d_bpk[:])

            def wsl(kind, i, j=0, rows=128):
                off = _w_off(kind, i, j)
                return wpk[:rows, off:off + C]

            def bap(kind, i=0):
                o = _b_off(kind, i)
                n = M_F1 if kind == "f1" else C
                return bpk[:n, o:o + 1]

            def wide_chunk(i, taps, T_out, write_out, c0, mul_pool=False):
                """One CH-column chunk of wide conv layer i."""
                N = min(CH, T_out - c0)
                Nmm = N + (N & 1)
                aps = ps.tile([C, Nmm], F32, tag="aps")
                gps = ps.tile([C, Nmm], F32, tag="gps")
                for half, pt in (("a", aps), ("g", gps)):
                    for j, (src, off) in enumerate(taps):
                        nc.tensor.matmul(
                            pt[:], wsl(half, i, j),
                            src[:, off + c0: off + c0 + Nmm],
                            start=(j == 0), stop=(j == 3))
                ta = sb.tile([C, Nmm], BF16, tag="ta", bufs=2)
                sg = sb.tile([C, Nmm], BF16, tag="sg", bufs=2)
                nc.scalar.activation(ta[:], aps[:], AF.Tanh, bias=bap("a", i))
                nc.scalar.activation(sg[:], gps[:], AF.Sigmoid, bias=bap("g", i))
                x2 = sb.tile([C, Nmm], BF16, tag="x2", bufs=2)
                # Pool for the fused L3 (DVE is busy with VQ there), DVE else
                if mul_pool:
                    nc.gpsimd.tensor_mul(x2[:], ta[:], sg[:])
                else:
                    nc.vector.tensor_mul(x2[:], ta[:], sg[:])
                xps = ps.tile([C, Nmm], F32, tag="xps")
                nc.tensor.matmul(xps[:], wsl("1", i, rows=C), x2[:],
                                 start=True, stop=True)
                write_out(c0, N, xps[:, :N])

            def encoder_units(s, g, is_query, vq_cb=None):
                """Chunk-level emitters for the scheduler.

                Layer 3 fuses the f0/f1 head (+VQ for search)."""
                T0h, T1 = g["T0h"], g["T1"]
                E1, O1 = g["E1"], g["O1"]
                T2, T3, T4 = g["T2"], g["T3"], g["T4"]
                d_e, d_o = (d_qe, d_qo) if is_query else (d_se, d_so)
                sfx = f"{'q' if is_query else 's'}{s}"
                st = {}

                def load():
                    x0e = sb.tile([128, T0h + 8], BF16, tag=f"x0e{sfx}")
                    x0o = sb.tile([128, T0h + 8], BF16, tag=f"x0o{sfx}")
                    nc.vector.memset(x0e[:, T0h:], 0.0)
                    nc.vector.memset(x0o[:, T0h:], 0.0)
                    for dst, dsrc in ((x0e, d_e), (x0o, d_o)):
                        for c0 in range(0, T0h, 2048):
                            n = min(2048, T0h - c0)
                            nc.sync.dma_start(dst[:, c0:c0 + n],
                                              dsrc[s, :, c0:c0 + n])
                    st.update(x0e=x0e, x0o=x0o)

                def alloc():
                    x1e = sb.tile([128, E1 + 8], BF16, tag=f"x1e{sfx}")
                    x1o = sb.tile([128, O1 + 8], BF16, tag=f"x1o{sfx}")
                    nc.vector.memset(x1e[:, E1:], 0.0)
                    nc.vector.memset(x1o[:, O1:], 0.0)
                    nc.sync.dma_start(x1e[C:, :E1], d_z[:, :E1])
                    nc.sync.dma_start(x1o[C:, :O1], d_z[:, :O1])
                    x2f = sb.tile([128, T2 + 8], BF16, tag=f"x2f{sfx}")
                    nc.vector.memset(x2f[:, T2:], 0.0)
                    nc.sync.dma_start(x2f[C:, :T2], d_z[:, :T2])
                    x3f = sb.tile([128, T3 + 8], BF16, tag=f"x3f{sfx}")
                    nc.vector.memset(x3f[:, T3:], 0.0)
                    nc.sync.dma_start(x3f[C:, :T3], d_z[:, :T3])
                    x4f = sb.tile([128, T4 + 8], BF16, tag=f"x4f{sfx}")
                    nc.vector.memset(x4f[:, T4:], 0.0)
                    nc.sync.dma_start(x4f[C:, :T4], d_z[:, :T4])
                    # f1(relu(f0(x))); rows 80/81 of enc are const 1.0
                    # (zero weights, bias 1) for the VQ score offsets.
                    enc = sb.tile([128, T4], BF16, tag=f"enc{sfx}")
                    nc.sync.dma_start(enc[M_F1:, :], d_z[:128 - M_F1, :T4])
                    st.update(x1e=x1e, x1o=x1o, x2f=x2f,
                              x3f=x3f, x4f=x4f, enc=enc)

                def w0(c0, N, xps):
                    ne, no = (N + 1) // 2, N // 2
                    h = c0 // 2
                    nc.scalar.activation(
                        st["x1e"][:C, h:h + ne], xps[:, 0:N:2], AF.Identity,
                        bias=bap("1", 0))
                    nc.vector.tensor_scalar(
                        st["x1o"][:C, h:h + no], xps[:, 1:N:2], bap("1", 0),
                        None, op0=OP.add)

                def w1(c0, N, xps):
                    nc.vector.scalar_tensor_tensor(
                        out=st["x2f"][:C, c0:c0 + N], in0=xps,
                        scalar=bap("1", 1),
                        in1=st["x1o"][:C, c0 + 1:c0 + 1 + N],
                        op0=OP.add, op1=OP.add)

                def w2(c0, N, xps):
                    nc.vector.scalar_tensor_tensor(
                        out=st["x3f"][:C, c0:c0 + N], in0=xps,
                        scalar=bap("1", 2),
                        in1=st["x2f"][:C, c0 + 3:c0 + 3 + N],
                        op0=OP.add, op1=OP.add)

                def w3(c0, N, xps):
                    # keep the x4f write off DVE (VQ reduces queue there):
                    # ACT applies the 1x1 bias, Pool adds the residual.
                    x4f, x3f, enc = st["x4f"], st["x3f"], st["enc"]
                    x4t = sb.tile([C, N], BF16, tag="x4t", bufs=2)
                    nc.scalar.activation(x4t[:], xps, AF.Identity,
                                         bias=bap("1", 3))
                    nc.gpsimd.tensor_add(x4f[:C, c0:c0 + N], x4t[:],
                                         x3f[:C, c0 + 3:c0 + 3 + N])
                    p0 = ps.tile([C, N], F32, tag="xps")
                    nc.tensor.matmul(p0[:], wsl("f0", 0), x4f[:, c0:c0 + N],
                                     start=True, stop=True)
                    xf = sb.tile([C, N], BF16, tag="xf", bufs=2)
                    nc.scalar.activation(xf[:], p0[:], AF.Relu, bias=bap("f0"))
                    p1 = ps.tile([M_F1, N], F32, tag="xps")
                    nc.tensor.matmul(p1[:], wpk[:C, 2960:2960 + M_F1], xf[:],
                                     start=True, stop=True)
                    nc.scalar.activation(enc[:M_F1, c0:c0 + N], p1[:],
                                         AF.Identity, bias=bap("f1"))
                    if vq_cb is not None:
                        vq_cb(c0, N, enc)

                Ts = [T1, T2, T3, T4]
                writers = [w0, w1, w2, w3]

                def taps(i):
                    if i == 0:
                        return [(st["x0e"], 0), (st["x0o"], 0),
                                (st["x0e"], 1), (st["x0o"], 1)]
                    if i == 1:
                        return [(st["x1e"], 0), (st["x1o"], 0),
                                (st["x1e"], 1), (st["x1o"], 1)]
                    x = st["x2f"] if i == 2 else st["x3f"]
                    return [(x, 0), (x, 1), (x, 2), (x, 3)]

                def emit(i, c):
                    wide_chunk(i, taps(i), Ts[i], writers[i], c * CH,
                               mul_pool=(i == 3))

                nch = [-(-t // CH) for t in Ts]
                ins = [E1, T2, T3]

                def req(i, c):
                    """Chunks of layer i-1 needed before chunk c of layer i."""
                    per = CH // 2 if i == 1 else CH
                    need = min(CH * c + 515, ins[i - 1])
                    return min(nch[i - 1], -(-need // per))

                return dict(load=load, alloc=alloc, emit=emit, nch=nch,
                            req=req, st=st)

            def run_sched(encs):
                """Greedy deepest-ready-first, round-robin over samples."""
                prog = [[0] * 4 for _ in encs]
                remaining = sum(sum(e["nch"]) for e in encs)
                turn = 0
                while remaining:
                    emitted = False
                    for k in range(len(encs)):
                        sidx = (turn + k) % len(encs)
                        e, p = encs[sidx], prog[sidx]
                        for i in (3, 2, 1, 0):
                            if p[i] >= e["nch"][i]:
                                continue
                            if i > 0 and p[i - 1] < e["req"](i, p[i]):
                                continue
                            e["emit"](i, p[i])
                            p[i] += 1
                            remaining -= 1
                            emitted = True
                            break
                        if emitted:
                            break
                    turn += 1
                    assert emitted, "scheduler deadlock"

            # ---- query encoders ----
            qencs = [encoder_units(s, GEO_QUERY, True) for s in range(SPC)]
            for e in qencs:
                e["load"]()        # input DMAs queue first
            for e in qencs:
                e["alloc"]()
            # remaining static tables arrive behind the query inputs
            nc.sync.dma_start(wpk[:, LBLK:], d_wpk[:, LBLK:])
            epk3 = sb.tile([128, 3 * NK], BF16, tag="epk3")
            nc.sync.dma_start(epk3[:], d_epk3[:])
            wlt = sb.tile([C, 2], BF16, tag="wlt")
            nc.sync.dma_start(wlt[:], d_wlt[:])
            brow = sb.tile([1, 4], F32, tag="brow")
            nc.sync.dma_start(brow[:], d_blt[:])
            run_sched(qencs)

            zred = sb.tile([128, 4], F32, tag="zred")
            vts = {}
            for s in range(SPC):
                enc_q = qencs[s]["st"]["enc"]
                vps = ps.tile([2, 504], F32, tag="xps")
                nc.tensor.matmul(vps[:], wlt[:], enc_q[:C, :504],
                                 start=True, stop=True)  # 80-row: tiny
                vrow = sb.tile([2, 2048], F32, tag="vrow", bufs=2)
                for k in range(4):
                    nc.scalar.activation(vrow[:, 504 * k:504 * (k + 1)],
                                         vps[:], AF.Copy)
                nc.vector.memset(vrow[:, 2016:2040], 0.0)
                nc.vector.memset(vrow[:, 2040:2048], NEG)
                nc.sync.dma_start(d_zbuf[2 * s:2 * s + 2, :], vrow[:])
                vt0 = sb.tile([128, 16], F32, tag=f"vt{2 * s}")
                vt1 = sb.tile([128, 16], F32, tag=f"vt{2 * s + 1}")
                nc.sync.dma_start(
                    vt0[:], d_zbuf[2 * s].rearrange("(b p) -> p b", p=128))
                nc.sync.dma_start(
                    vt1[:], d_zbuf[2 * s + 1].rearrange("(b p) -> p b", p=128))
                vts[s] = (vt0, vt1)

            # ---- search encoders with fused VQ, interleaved ----
            T4s = GEO_SEARCH["T4"]
            accs = {}
            for s in range(SPC):
                mt = sb.tile([128, 16], F32, tag=f"mt{s}")
                u0t = sb.tile([128, 16], F32, tag=f"u0t{s}")
                u1t = sb.tile([128, 16], F32, tag=f"u1t{s}")
                nc.vector.memset(mt[:], NEG)
                nc.vector.memset(u0t[:], NEG)
                nc.vector.memset(u1t[:], NEG)
                accs[s] = (mt, u0t, u1t)

            def make_vq_cb(s):
                mt, u0t, u1t = accs[s]

                def vq_blocks(c0, N, enc):
                    b0 = (c0 + 127) // 128
                    b1 = (c0 + N) // 128 if c0 + N < T4s else 16
                    for b in range(b0, b1):
                        t0 = 128 * b
                        P = min(128, T4s - t0)
                        for ti, tgt in ((0, mt), (1, u0t), (2, u1t)):
                            sps = ps.tile([P, NK], F32, tag="vq")
                            nc.tensor.matmul(
                                sps[:], enc[:, t0:t0 + P],
                                epk3[:, NK * ti:NK * (ti + 1)],
                                start=True, stop=True)
                            nc.vector.tensor_reduce(
                                tgt[:P, b:b + 1], sps[:], axis=AX.X,
                                op=OP.max)
                return vq_blocks

            sencs = [encoder_units(s, GEO_SEARCH, False,
                                   vq_cb=make_vq_cb(s))
                     for s in range(SPC)]
            for e in sencs:
                e["load"]()
            for e in sencs:
                e["alloc"]()
            run_sched(sencs)

            # ---- z = (u - m)*BIG + vt, reduce ----
            for s in range(SPC):
                mt, u0t, u1t = accs[s]
                for j, ut in ((0, u0t), (1, u1t)):
                    zt = sb.tile([128, 16], F32, tag="zt", bufs=2)
                    nc.vector.tensor_sub(zt[:], ut[:], mt[:])
                    nc.vector.scalar_tensor_tensor(
                        out=zt[:], in0=zt[:], scalar=BIG, in1=vts[s][j][:],
                        op0=OP.mult, op1=OP.add)
                    nc.vector.tensor_reduce(
                        zred[:, 2 * s + j:2 * s + j + 1], zt[:], axis=AX.X,
                        op=OP.max)

            import concourse.bass_isa as bass_isa
            zar = sb.tile([128, 4], F32, tag="zar")
            nc.gpsimd.partition_all_reduce(zar[:], zred[:], channels=128,
                                           reduce_op=bass_isa.ReduceOp.max)
            zrow = sb.tile([1, 4], F32, tag="zrow")
            nc.vector.tensor_add(zrow[:], zar[0:1, :], brow[:])
            outv = sb.tile([1, 4], F32, tag="outv")
            nc.scalar.activation(outv[:], zrow[:], AF.Tanh)
            nc.sync.dma_start(d_out[:], outv[:])

    nc.finalize()
    return nc


_NC_CACHE = None


def _get_nc():
    global _NC_CACHE
    if _NC_CACHE is None:
        _NC_CACHE = _build()
    return _NC_CACHE


def prep_inputs(search, query, w_wide, b_wide, w_1x1, b_1x1, w_f0, b_f0,
                w_f1, b_f1, embedding, w_lin, b_lin):
    """Host-side packing -> list of per-core input maps (bf16 operands)."""
    f = np.float32
    bf = ml_dtypes.bfloat16
    search = np.asarray(search, f)
    query = np.asarray(query, f)

    def pad128(a):
        # (N, T, C) -> (N, 128, T) channel-major, rows C..127 zero
        n, t, _ = a.shape
        out = np.zeros((n, 128, t), dtype=bf)
        out[:, :C, :] = a.transpose(0, 2, 1).astype(bf)
        return out

    se = pad128(search[:, 0::2, :])
    so = pad128(search[:, 1::2, :])
    qe = pad128(query[:, 0::2, :])
    qo = pad128(query[:, 1::2, :])

    w_wide = np.asarray(w_wide, f)
    cols = []
    for i in range(4):
        for j in range(4):
            cols.append(w_wide[i, :C, :, j].T)     # a taps
        for j in range(4):
            cols.append(w_wide[i, C:, :, j].T)     # g taps
        cols.append(np.asarray(w_1x1, f)[i, :, :, 0].T)
    cols.append(np.asarray(w_f0, f)[:, :, 0].T)
    wf1 = np.zeros((C, M_F1), f)
    wf1[:, :C] = np.asarray(w_f1, f)[:, :, 0].T   # cols 80/81 stay zero
    cols.append(wf1)
    wpk80 = np.ascontiguousarray(np.concatenate(cols, axis=1))
    assert wpk80.shape == (C, WPACK_COLS)
    wpk = np.zeros((128, WPACK_COLS), dtype=bf)
    wpk[:C] = wpk80.astype(bf)

    b_wide = np.asarray(b_wide, f)
    bcols = [b_wide[i, :C] for i in range(4)]
    bcols += [b_wide[i, C:] for i in range(4)]
    bcols += [np.asarray(b_1x1, f)[i] for i in range(4)]
    bcols += [np.asarray(b_f0, f), np.asarray(b_f1, f)]
    bpk = np.zeros((M_F1, 14), f)
    bpk[:C] = np.stack(bcols, axis=1)
    bpk[C, _b_off("f1")] = 1.0     # f1 rows 80/81 = 0*x + 1.0 -> const-1
    bpk[C + 1, _b_off("f1")] = 1.0

    emb = np.asarray(embedding, f)[0]            # (512, 80)
    e2 = (emb.astype(np.float64) ** 2).sum(1)
    ew = (emb.astype(np.float64) @ np.asarray(w_lin, f).T.astype(np.float64))
    epk3 = np.zeros((128, 3 * NK), f)
    for ti in range(3):
        epk3[:C, NK * ti:NK * (ti + 1)] = emb.T
        epk3[C, NK * ti:NK * (ti + 1)] = -0.5 * e2
    epk3[C + 1, NK:2 * NK] = ew[:, 0] / BIG
    epk3[C + 1, 2 * NK:3 * NK] = ew[:, 1] / BIG
    epk3 = epk3.astype(bf)
    z48 = np.zeros((48, 4104), dtype=bf)
    wlt = np.ascontiguousarray(np.asarray(w_lin, f).T).astype(bf)
    b_lin = np.asarray(b_lin, f)
    blt = np.array([[b_lin[0], b_lin[1], b_lin[0], b_lin[1]]], f)

    maps = []
    for c in range(NCORES):
        sl = slice(SPC * c, SPC * (c + 1))
        maps.append({
            "se": se[sl], "so": so[sl], "qe": qe[sl], "qo": qo[sl],
            "wpk": wpk, "bpk": bpk, "epk3": epk3, "wlt": wlt, "blt": blt,
            "z48": z48,
        })
    return maps


def kernel(**inputs):
    nc = _get_nc()
    maps = prep_inputs(**inputs)
    res = run_bass_kernel_spmd(nc, maps, core_ids=list(range(NCORES)))
    out = np.concatenate([r["out"].reshape(SPC, 2) for r in res.results],
                         axis=0)
    return out.astype(np.float32)


if __name__ == "__main__":
    import reference
    inputs = {k: np.asarray(v) for k, v in reference.setup_inputs().items()}
    got = kernel(**inputs)
    print(got)
